# revision 2
# baseline (speedup 1.0000x reference)
"""Trainium2 Bass kernel for nn_Complex2LayerMAPGraphConvolution (v2).

Complex-weighted 2-layer graph convolution + linear head on 8 NeuronCores
with edge-cut (destination-row-block) graph parallelism.

v2 strategy (vs v1): the graph is known at kernel-call time, so all
index-dependent expansion moves to the host and the device only streams:
  - layer-1 edge source features are pre-gathered on the HOST into a dense
    [128, NCH, 128] f16 stream (no dma_gather, no GpSimd for layer 1).
  - the weighted one-hot scatter masks ((iota==lrow)*wr | *wi including the
    cos/sin edge phase) are precomputed on the HOST as a [128, NCH, 256] f16
    array and streamed from HBM for BOTH layers (no DVE mask building).
  - layer 2 still uses gpsimd.dma_gather on the AllGather'd hidden table
    (runtime data), which is now the only GpSimd work and no longer
    contends with DVE for the shared SBUF port.

Per core (owns N/8 destination nodes), per chunk of 128 edges: TensorE
computes G.T @ [Mr|Mi] accumulating all 4 complex spmm products in PSUM per
destination block; per block: FC + complex recombination via two stacked
weight matmuls, ReLU+bias on ScalarE; layer-1 output transposed to
node-major f16 and AllGather'd so layer 2 can gather any source's features;
linear head fused per block off the layer-2 tile.
"""

import os
import sys

for _p in ("/opt/trn_rl_repo", "/root/.axon_site/_ro/trn_rl_repo"):
    if os.path.isdir(_p) and _p not in sys.path:
        sys.path.insert(0, _p)

import numpy as np

import concourse.bass as bass
import concourse.tile as tile
from concourse import mybir, bacc
from concourse.masks import make_identity

P = 128
F16 = mybir.dt.float16
F32 = mybir.dt.float32
I16 = mybir.dt.int16


class Cfg:
    def __init__(self, n_nodes, n_edges, cores=8, gk=4, sg=3, rsz=25000,
                 gks=16):
        assert n_nodes % cores == 0
        self.N = n_nodes
        self.E = n_edges
        self.CORES = cores
        self.NPC = n_nodes // cores            # nodes per core
        self.NB = (self.NPC + P - 1) // P      # dest blocks per core
        self.NV_LAST = self.NPC - (self.NB - 1) * P
        self.GK = gk                           # max chunks per gather call
        self.SG = sg                           # blocks per supergroup
        self.GKS = gks                         # chunks per stream tile
        self.RSZ = min(rsz, n_nodes)           # rows per index range
        self.NR = (n_nodes + self.RSZ - 1) // self.RSZ
        assert self.RSZ <= 32767


def host_prep(cfg, real, imag, ew, q, ent, ccf, W1, b1, W2, b2, W3, b3,
              row, col):
    """Pure index/layout preprocessing (sharding) + weight layout prep."""
    N, E, C, NPC, NB = cfg.N, cfg.E, cfg.CORES, cfg.NPC, cfg.NB
    NR, RSZ, SG = cfg.NR, cfg.RSZ, cfg.SG

    core = row // NPC
    r_local = row - core * NPC
    blk = r_local // P
    lrow = (r_local - blk * P).astype(np.int64)
    # piece-interleaved table layout: piece p holds rows c*QS+q for node
    # c*NPC + p*QS + q, so each AllGather piece is a contiguous slice of
    # every core's layer-1 output and int16 indices stay < 8*QS = 25000
    QS = NPC // NR
    src_c = col // NPC
    src_l = col - src_c * NPC
    rid = src_l // QS

    # segment sizes equalized across cores; +1 guarantees >=1 trailing pad
    cnt = np.zeros((C, NB, NR), np.int64)
    np.add.at(cnt, (core, blk, rid), 1)
    seg_cpb = -(-(cnt.max(axis=0) + 1) // P)           # [NB, NR] chunks

    # chunk numbering: for supergroup g: for r: for b in g: seg(b, r)
    n_groups = (NB + SG - 1) // SG
    seg_start = np.zeros((NB, NR), np.int64)
    calls = []          # (start_chunk, n_chunks, range_id)
    block_spans = {}    # b -> list of (c0, c1) in chunk order (per r)
    nch = 0
    for g in range(n_groups):
        bs = list(range(g * SG, min((g + 1) * SG, NB)))
        for r in range(NR):
            span0 = nch
            for b in bs:
                seg_start[b, r] = nch
                block_spans.setdefault(b, []).append(
                    (nch, nch + int(seg_cpb[b, r])))
                nch += int(seg_cpb[b, r])
            c0 = span0
            while c0 < nch:
                w = min(cfg.GK, nch - c0)
                calls.append((c0, w, r))
                c0 += w
    NCH = nch

    # edge -> (core, chunk, partition)
    key = (core.astype(np.int64) * NB + blk) * NR + rid
    order = np.argsort(key, kind="stable")
    ks = key[order]
    starts = np.searchsorted(ks, np.arange(C * NB * NR))
    rank = np.arange(E) - starts[ks]
    c_ = ks // (NB * NR)
    b_ = (ks // NR) % NB
    r_ = ks % NR
    chunk = seg_start[b_, r_] + rank // P
    part = rank % P
    e = order

    # host edge weights: wr = ew*cos(q*(ent+ccf)), wi = ew*sin(...)
    phase = np.float32(q) * (ent + ccf)
    wr = (ew * np.cos(phase)).astype(np.float32)
    wi = (ew * np.sin(phase)).astype(np.float32)
    lrow_f = lrow.astype(np.float32)

    # int16 gather indices for layer 2: position (chunk*128+part) ->
    # [pos%16, pos//16], replicated across the 8 16-partition groups
    gidxA = np.zeros((C, 16, NCH * 8), np.int16)
    pos = chunk * P + part
    piece_idx = src_c * QS + (src_l - rid * QS)        # index within piece
    gidxA[c_, pos % 16, pos // 16] = piece_idx[e].astype(np.int16)
    gidxA = np.tile(gidxA, (1, 8, 1))                  # [C, 128, NCH*8]

    tab = np.concatenate([real, imag], axis=1).astype(np.float16)  # [N, 128]

    def stk_a(W):
        H, Fd = W.shape
        out = np.zeros((2 * Fd, 2 * H), np.float16)
        out[:Fd, :H] = W.T
        out[Fd:, H:] = W.T
        return out

    def stk_b(W):
        H, Fd = W.shape
        out = np.zeros((2 * Fd, 2 * H), np.float16)
        out[Fd:, :H] = -W.T
        out[:Fd, H:] = W.T
        return out

    def brow(b):
        out = np.zeros((2 * len(b), 1), np.float32)
        out[len(b):, 0] = 2.0 * b
        return out

    consts = {
        "wa1": stk_a(W1), "wb1": stk_b(W1), "brow1": brow(b1),
        "wa2": stk_a(W2), "wb2": stk_b(W2), "brow2": brow(b2),
        "w3s": W3.T.astype(np.float16).copy(),           # [2H, O]
        "b3col": b3.astype(np.float32).reshape(-1, 1).copy(),
    }
    in_maps = []
    for c in range(cfg.CORES):
        m_e = c_ == c                        # edges of this core
        pa, ch = part[m_e], chunk[m_e]
        lr_c, ee = lrow[e][m_e], e[m_e]
        # weighted one-hot scatter masks [128, NCH, 256] f16
        maskc = np.zeros((P, NCH, 256), np.float16)
        maskc[pa, ch, lr_c] = wr[ee]
        maskc[pa, ch, P + lr_c] = wi[ee]
        # pre-gathered layer-1 edge source features [128, NCH, 128] f16
        g1c = np.zeros((P, NCH, P), np.float16)
        g1c[pa, ch, :] = tab[col[ee]]
        # compact per-chunk columns for on-device (DVE) mask building
        lwc = np.zeros((P, 3, NCH), np.float32)
        lwc[pa, 0, ch] = lrow_f[ee]
        lwc[pa, 1, ch] = wr[ee]
        lwc[pa, 2, ch] = wi[ee]
        m = {"g1": g1c, "mask": maskc, "gidx": gidxA[c], "lwc": lwc}
        m.update(consts)
        in_maps.append(m)
    meta = {"NCH": NCH, "calls": calls, "block_spans": block_spans,
            "n_groups": n_groups, "seg_cpb": seg_cpb}
    return in_maps, meta


def build_nc(cfg, meta):
    N, NPC, NB, GK, SG, NR, RSZ, GKS = (cfg.N, cfg.NPC, cfg.NB, cfg.GK,
                                        cfg.SG, cfg.NR, cfg.RSZ, cfg.GKS)
    NCH = meta["NCH"]
    calls = meta["calls"]
    block_spans = meta["block_spans"]
    n_groups = meta["n_groups"]
    O = 16
    NST = (NCH + GKS - 1) // GKS            # number of stream tiles
    NQ = int(os.environ.get('GNN_NQ', '4'))
    PREP = int(os.environ.get('GNN_PREP', '0'))
    W = int(os.environ.get('GNN_W', '128'))  # gather tile window (pool bufs)
    SCR = int(os.environ.get('GNN_SCR', '16384'))
    nc = bacc.Bacc(num_devices=cfg.CORES, num_swdge_queues=NQ,
                   dynamic_dma_scratch_size=SCR)

    g1_d = nc.declare_dram_parameter("g1", [P, NCH, P], F16, isOutput=False)
    mask_d = nc.declare_dram_parameter("mask", [P, NCH, 256], F16,
                                       isOutput=False)
    gidx_d = nc.declare_dram_parameter("gidx", [P, NCH * 8], I16,
                                       isOutput=False)
    lwc_d = nc.declare_dram_parameter("lwc", [P, 3, NCH], F32, isOutput=False)
    wa_d = [nc.declare_dram_parameter("wa1", [P, P], F16, isOutput=False),
            nc.declare_dram_parameter("wa2", [P, P], F16, isOutput=False)]
    wb_d = [nc.declare_dram_parameter("wb1", [P, P], F16, isOutput=False),
            nc.declare_dram_parameter("wb2", [P, P], F16, isOutput=False)]
    brow_d = [nc.declare_dram_parameter("brow1", [P, 1], F32, isOutput=False),
              nc.declare_dram_parameter("brow2", [P, 1], F32, isOutput=False)]
    w3s_d = nc.declare_dram_parameter("w3s", [P, O], F16, isOutput=False)
    b3_d = nc.declare_dram_parameter("b3col", [O, 1], F32, isOutput=False)
    out_t = nc.declare_dram_parameter("out_t", [O, NPC], F32, isOutput=True)

    QS = NPC // NR
    tab2in = [nc.dram_tensor(f"tab2in{p}", [QS, P], F16) for p in range(NR)]
    tabp = [nc.dram_tensor(f"tab2f{p}", [cfg.CORES * QS, P], F16,
                           addr_space="Shared") for p in range(NR)]
    # supergroup after which piece p of the layer-1 output is complete
    ag_sg = [(-(-((p + 1) * QS) // P) - 1) // SG for p in range(NR)]

    AluOp = mybir.AluOpType
    Act = mybir.ActivationFunctionType

    with tile.TileContext(nc) as tc:
        import contextlib
        with contextlib.ExitStack() as ctx:
            singles = ctx.enter_context(tc.tile_pool(name="singles", bufs=1))
            gpool = ctx.enter_context(tc.tile_pool(
                name="gpool", bufs=(W if PREP else 6)))
            mspool = ctx.enter_context(tc.tile_pool(name="mspool", bufs=4))
            g1pool = ctx.enter_context(tc.tile_pool(name="g1pool", bufs=4))
            p2pool = ctx.enter_context(tc.tile_pool(name="p2pool", bufs=2))
            lopool = ctx.enter_context(tc.tile_pool(name="lopool", bufs=2))
            twpool = ctx.enter_context(tc.tile_pool(name="twpool", bufs=2))
            topool = ctx.enter_context(tc.tile_pool(name="topool", bufs=2))
            pp_s = ctx.enter_context(tc.tile_pool(name="pp_s", bufs=4, space="PSUM"))
            pp_l = ctx.enter_context(tc.tile_pool(name="pp_l", bufs=2, space="PSUM"))
            pp_x = ctx.enter_context(tc.tile_pool(name="pp_x", bufs=2, space="PSUM"))

            # ---- resident metadata + constants ----
            gidx_s = singles.tile([P, NCH * 8], I16)
            nc.sync.dma_start(out=gidx_s, in_=gidx_d[:, :])
            DVEF = float(os.environ.get('GNN_DVEF', '0.35'))
            lwc_s = singles.tile([P, 3, NCH], F32)
            nc.sync.dma_start(out=lwc_s, in_=lwc_d[:, :, :])
            iota = singles.tile([P, P], F16)
            nc.gpsimd.iota(iota, pattern=[[1, P]], base=0,
                           channel_multiplier=0,
                           allow_small_or_imprecise_dtypes=True)

            wa = [singles.tile([P, P], F16, name=f"wa{i}") for i in range(2)]
            wb = [singles.tile([P, P], F16, name=f"wb{i}") for i in range(2)]
            brow = [singles.tile([P, 1], F32, name=f"brow{i}") for i in range(2)]
            for i in range(2):
                nc.sync.dma_start(out=wa[i], in_=wa_d[i][:, :])
                nc.sync.dma_start(out=wb[i], in_=wb_d[i][:, :])
                nc.sync.dma_start(out=brow[i], in_=brow_d[i][:, :])
            w3s = singles.tile([P, O], F16)
            nc.sync.dma_start(out=w3s, in_=w3s_d[:, :])
            b3c = singles.tile([O, 1], F32)
            nc.sync.dma_start(out=b3c, in_=b3_d[:, :])

            ident = singles.tile([P, P], F16)
            make_identity(nc, ident)

            # chunk -> (call index, offset within call)
            chunk_call = {}
            for ci, (c0, w, r) in enumerate(calls):
                for j in range(w):
                    chunk_call[c0 + j] = (ci, j)

            # Manual prep/trigger/wait protocol for layer-2 gathers:
            # descriptors are generated (prepare_only) ahead of the
            # AllGather data dependency; triggers fire per-call (count=1)
            # with at most PF calls of prefetch; consumers wait per-call
            # DMA-completion sems rotating over R sems (PF < R keeps the
            # cumulative 16-inc-per-call count an exact completion proof).
            R = 16
            PF = 12
            gsems = [nc.alloc_semaphore(f"gsem{i}") for i in range(R)]
            psems = [nc.alloc_semaphore(f"psem{q}") for q in range(NQ)]
            st = {"emitted": 0, "triggered": 0, "qpos": [0] * NQ,
                  "qfired": [0] * NQ}

            def emit_gather(g_tiles, prep):
                ci = st["emitted"]
                c0, w, r = calls[ci]
                q = ci % NQ
                gt = gpool.tile([P, GK, P], F16, tag="g", name=f"g_{ci}")
                g_tiles[ci] = gt
                ins = nc.gpsimd.dma_gather(
                    out_ap=gt[:, :w, :],
                    in_ap=tabp[r][0:, :],
                    idxs_ap=gidx_s[:, c0 * 8:(c0 + w) * 8],
                    num_idxs=w * P, num_idxs_reg=w * P,
                    elem_size=P, queue_num=q,
                    single_packet=os.environ.get('GNN_SP', '1') == '1',
                    prepare_only=bool(prep),
                    sem=gsems[ci % R] if prep else None)
                if prep:
                    ins.then_inc(psems[q], 1)
                st["qpos"][q] += 1
                st["emitted"] += 1

            def emit_trigger():
                ci = st["triggered"]
                q = ci % NQ
                st["qfired"][q] += 1
                nc.gpsimd.wait_ge(psems[q], st["qfired"][q])
                nc.gpsimd.trigger_dma(count=1, queue_num=q)
                st["triggered"] += 1

            # ---- two graph-conv layers ----
            for L in range(2):
                g_tiles = {}        # layer-2 gather tiles by call index
                ms_tiles = {}       # mask stream tiles by stream index
                g1_tiles = {}       # layer-1 G stream tiles by stream index

                def need_stream(c):
                    si = c // GKS
                    if si not in ms_tiles:
                        w = min(GKS, NCH - si * GKS)
                        mt = mspool.tile([P, GKS, 256], F16, tag="ms",
                                         name=f"ms{L}_{si}")
                        ms_tiles[si] = mt
                        if L == 0 and int((si + 1) * DVEF) > int(si * DVEF):
                            # build this tile's masks on the (idle) DVE:
                            # (iota==lrow)*wr | (iota==lrow)*wi per chunk
                            for cc in range(si * GKS, si * GKS + w):
                                j = cc - si * GKS
                                nc.vector.tensor_scalar(
                                    out=mt[:, j, 0:P], in0=iota[:, :],
                                    scalar1=lwc_s[:, 0, cc:cc + 1],
                                    scalar2=lwc_s[:, 1, cc:cc + 1],
                                    op0=AluOp.is_equal, op1=AluOp.mult)
                                nc.vector.tensor_scalar(
                                    out=mt[:, j, P:256], in0=iota[:, :],
                                    scalar1=lwc_s[:, 0, cc:cc + 1],
                                    scalar2=lwc_s[:, 2, cc:cc + 1],
                                    op0=AluOp.is_equal, op1=AluOp.mult)
                        else:
                            nc.sync.dma_start(
                                out=mt[:, :w, :],
                                in_=mask_d[:, si * GKS:si * GKS + w, :])
                        if L == 0:
                            g1t = g1pool.tile([P, GKS, P], F16, tag="g1",
                                              name=f"g1_{si}")
                            g1_tiles[si] = g1t
                            nc.scalar.dma_start(
                                out=g1t[:, :w, :],
                                in_=g1_d[:, si * GKS:si * GKS + w, :])
                    return si

                for g in range(n_groups):
                    bs = list(range(g * SG, min((g + 1) * SG, NB)))
                    first_chunk = block_spans[bs[0]][0][0]
                    last_chunk = block_spans[bs[-1]][-1][1]
                    if L == 1:
                        # gather calls: with PREP, emit prepare_only descs
                        # running AHEAD of consumption (window W tiles),
                        # triggers paced PF calls ahead of use
                        last_ci = max(ci for ci, (c0, w, r) in enumerate(calls)
                                      if first_chunk <= c0 < last_chunk)
                        GATE = int(os.environ.get('GNN_GATE', '3'))
                        if GATE >= 0 and st["emitted"] == 0:
                            # delay all desc-gen until piece GATE lands to
                            # keep Q7 off the SBUF port while DVE builds
                            gw = singles.tile([1, 2], F16, name="gw")
                            nc.gpsimd.dma_start(out=gw,
                                                in_=tabp[GATE][0:1, 0:2])
                        if PREP:
                            first = st["emitted"] == 0
                            tgt = min(len(calls), last_ci + 1 + (W - 32))
                            while st["emitted"] < tgt:
                                emit_gather(g_tiles, 1)
                            if first:
                                # tiny Pool-engine read of tab2f after the
                                # first prep batch: picks up the AllGather
                                # sync dep so every trigger (Pool,
                                # in-order) fires after the data arrives
                                agw = singles.tile([1, 2], F16, name="agw")
                                nc.gpsimd.dma_start(out=agw,
                                                    in_=tabp[NR - 1][0:1, 0:2])
                            tgt = min(st["emitted"], last_ci + 1 + PF)
                            while st["triggered"] < tgt:
                                emit_trigger()
                        else:
                            while st["emitted"] < last_ci + 1:
                                emit_gather(g_tiles, 0)
                    # one PSUM bank per block (sim tracks accumulation
                    # groups per bank; sharing a bank corrupts them)
                    pair = {}
                    for k in range(len(bs)):
                        pair[k] = pp_s.tile([P, 256], F32, space="PSUM",
                                            tag="ps", name=f"ps{L}_{g}_{k}")
                    blk_of = {}
                    blk_first = {}
                    blk_last = {}
                    for bi, b in enumerate(bs):
                        spans = block_spans[b]
                        blk_first[b] = spans[0][0]
                        blk_last[b] = spans[-1][1] - 1
                        for (c0, c1) in spans:
                            for c in range(c0, c1):
                                blk_of[c] = (bi, b)
                    for c in sorted(blk_of):
                        bi, b = blk_of[c]
                        psum = pair[bi]
                        si = need_stream(c)
                        rhs = ms_tiles[si][:, c - si * GKS, :]
                        if L == 0:
                            lhsT = g1_tiles[si][:, c - si * GKS, :]
                        else:
                            ci, j = chunk_call[c]
                            if PREP and j == 0:
                                # data-landing gate for this call's tile
                                nc.tensor.wait_ge(gsems[ci % R],
                                                  16 * (ci // R + 1))
                            lhsT = g_tiles[ci][:, j, :]
                        nc.tensor.matmul(
                            psum[:, 0:256],
                            lhsT=lhsT, rhs=rhs,
                            start=(c == blk_first[b]), stop=(c == blk_last[b]),
                            skip_group_check=True)
                    # finalize blocks
                    for bi, b in enumerate(bs):
                        psum = pair[bi]
                        p2c = p2pool.tile([P, 256], F16, tag="p2",
                                          name=f"p2_{L}_{b}")
                        nc.scalar.activation(out=p2c, in_=psum[:, 0:256],
                                             func=Act.Copy)
                        psl = pp_l.tile([P, P], F32, space="PSUM", tag="pl",
                                        name=f"pl{L}_{b}")
                        nc.tensor.matmul(psl[:, :], lhsT=wa[L], rhs=p2c[:, 0:P],
                                         start=True, stop=False)
                        nc.tensor.matmul(psl[:, :], lhsT=wb[L],
                                         rhs=p2c[:, P:256],
                                         start=False, stop=True)
                        lout = lopool.tile([P, P], F16, tag="lo",
                                           name=f"lo{L}_{b}")
                        nc.scalar.activation(out=lout, in_=psl, func=Act.Relu,
                                             bias=brow[L][:, 0:1])
                        nv = P if b < NB - 1 else cfg.NV_LAST
                        if L == 0:
                            pst = pp_x.tile([P, P], F16, space="PSUM",
                                            tag="px", name=f"px{b}")
                            nc.tensor.transpose(pst[:, :], lout[:, :],
                                                ident[:, :])
                            tblw = twpool.tile([P, P], F16, tag="tw",
                                               name=f"tw{b}")
                            nc.vector.tensor_copy(out=tblw, in_=pst)
                            # write node-major rows into piece tensors,
                            # splitting blocks that straddle a boundary
                            r0 = b * P
                            while r0 < b * P + nv:
                                p = r0 // QS
                                r1 = min(b * P + nv, (p + 1) * QS)
                                nc.sync.dma_start(
                                    out=tab2in[p][r0 - p * QS:r1 - p * QS, :],
                                    in_=tblw[r0 - b * P:r1 - b * P, :])
                                r0 = r1
                        else:
                            pso = pp_x.tile([P, P], F32, space="PSUM",
                                            tag="px", name=f"pxo{b}")
                            nc.tensor.matmul(pso[:O, :], lhsT=w3s[:, :],
                                             rhs=lout[:, :], start=True,
                                             stop=True)
                            osb = topool.tile([O, P], F32, tag="to",
                                              name=f"to{b}")
                            nc.scalar.activation(out=osb, in_=pso[:O, :],
                                                 func=Act.Identity,
                                                 bias=b3c[:, 0:1])
                            nc.sync.dma_start(out=out_t[:, b * P:b * P + nv],
                                              in_=osb[:, :nv])
                    if L == 0:
                        # fire each piece's AllGather as soon as its blocks
                        # are finalized — pieces 0..NR-2 overlap layer-1
                        # compute, and layer-2 desc-gen for early pieces
                        # can start before layer 1 finishes
                        for p in range(NR):
                            if ag_sg[p] == g:
                                nc.gpsimd.collective_compute(
                                    "AllGather", AluOp.bypass,
                                    replica_groups=[list(range(cfg.CORES))],
                                    ins=[tab2in[p].ap().opt()],
                                    outs=[tabp[p].ap().opt()],
                                )
    nc.compile()
    return nc


_CACHE = {}


def _get_nc(cfg, meta):
    key = (cfg.N, cfg.E, cfg.CORES, cfg.GK, cfg.SG, cfg.GKS,
           tuple(c for call in meta["calls"] for c in call))
    if key not in _CACHE:
        _CACHE[key] = build_nc(cfg, meta)
    return _CACHE[key]


def run(cfg, inputs, trace=False):
    from concourse.bass_utils import run_bass_kernel_spmd

    in_maps, meta = host_prep(
        cfg,
        np.asarray(inputs["real_feature"], np.float32),
        np.asarray(inputs["imag_feature"], np.float32),
        np.asarray(inputs["edge_weight_sym"], np.float32),
        np.float32(inputs["exp_weight_q"]),
        np.asarray(inputs["edge_entropy"], np.float32),
        np.asarray(inputs["edge_cluster_coefficient"], np.float32),
        np.asarray(inputs["W1"], np.float32), np.asarray(inputs["b1"], np.float32),
        np.asarray(inputs["W2"], np.float32), np.asarray(inputs["b2"], np.float32),
        np.asarray(inputs["W3"], np.float32), np.asarray(inputs["b3"], np.float32),
        np.asarray(inputs["row"]).astype(np.int64),
        np.asarray(inputs["col"]).astype(np.int64),
    )
    nc = _get_nc(cfg, meta)
    res = run_bass_kernel_spmd(nc, in_maps, list(range(cfg.CORES)), trace=trace)
    out = np.empty((cfg.N, 16), np.float32)
    for c in range(cfg.CORES):
        out[c * cfg.NPC:(c + 1) * cfg.NPC, :] = res.results[c]["out_t"].T
    return out, res


def kernel(**inputs) -> np.ndarray:
    cfg = Cfg(100000, 1000000, cores=8,
              gk=int(os.environ.get('GNN_GK', '3')),
              gks=int(os.environ.get('GNN_GKS', '24')))
    out, _ = run(cfg, inputs, trace=False)
    return out


# revision 3
# speedup vs baseline: 1.1093x; 1.1093x over previous
"""Trainium2 Bass kernel for nn_Complex2LayerMAPGraphConvolution (v2).

Complex-weighted 2-layer graph convolution + linear head on 8 NeuronCores
with edge-cut (destination-row-block) graph parallelism.

v2 strategy (vs v1): the graph is known at kernel-call time, so all
index-dependent expansion moves to the host and the device only streams:
  - layer-1 edge source features are pre-gathered on the HOST into a dense
    [128, NCH, 128] f16 stream (no dma_gather, no GpSimd for layer 1).
  - the weighted one-hot scatter masks ((iota==lrow)*wr | *wi including the
    cos/sin edge phase) are precomputed on the HOST as a [128, NCH, 256] f16
    array and streamed from HBM for BOTH layers (no DVE mask building).
  - layer 2 still uses gpsimd.dma_gather on the AllGather'd hidden table
    (runtime data), which is now the only GpSimd work and no longer
    contends with DVE for the shared SBUF port.

Per core (owns N/8 destination nodes), per chunk of 128 edges: TensorE
computes G.T @ [Mr|Mi] accumulating all 4 complex spmm products in PSUM per
destination block; per block: FC + complex recombination via two stacked
weight matmuls, ReLU+bias on ScalarE; layer-1 output transposed to
node-major f16 and AllGather'd so layer 2 can gather any source's features;
linear head fused per block off the layer-2 tile.
"""

import os
import sys

for _p in ("/opt/trn_rl_repo", "/root/.axon_site/_ro/trn_rl_repo"):
    if os.path.isdir(_p) and _p not in sys.path:
        sys.path.insert(0, _p)

import numpy as np

import concourse.bass as bass
import concourse.tile as tile
from concourse import mybir, bacc
from concourse.masks import make_identity

P = 128
F16 = mybir.dt.float16
F32 = mybir.dt.float32
I16 = mybir.dt.int16


class Cfg:
    def __init__(self, n_nodes, n_edges, cores=8, gk=4, sg=3, rsz=25000,
                 gks=16):
        assert n_nodes % cores == 0
        self.N = n_nodes
        self.E = n_edges
        self.CORES = cores
        self.NPC = n_nodes // cores            # nodes per core
        self.NB = (self.NPC + P - 1) // P      # dest blocks per core
        self.NV_LAST = self.NPC - (self.NB - 1) * P
        self.GK = gk                           # max chunks per gather call
        self.SG = sg                           # blocks per supergroup
        self.GKS = gks                         # chunks per stream tile
        self.RSZ = min(rsz, n_nodes)           # rows per index range
        self.NR = (n_nodes + self.RSZ - 1) // self.RSZ
        assert self.RSZ <= 32767


def host_prep(cfg, real, imag, ew, q, ent, ccf, W1, b1, W2, b2, W3, b3,
              row, col):
    """Pure index/layout preprocessing (sharding) + weight layout prep."""
    N, E, C, NPC, NB = cfg.N, cfg.E, cfg.CORES, cfg.NPC, cfg.NB
    NR, RSZ, SG = cfg.NR, cfg.RSZ, cfg.SG

    core = row // NPC
    r_local = row - core * NPC
    blk = r_local // P
    lrow = (r_local - blk * P).astype(np.int64)
    # piece-interleaved table layout: piece p holds rows c*QS+q for node
    # c*NPC + p*QS + q, so each AllGather piece is a contiguous slice of
    # every core's layer-1 output and int16 indices stay < 8*QS = 25000
    QS = NPC // NR
    src_c = col // NPC
    src_l = col - src_c * NPC
    rid = src_l // QS

    # segment sizes equalized across cores; +1 guarantees >=1 trailing pad
    cnt = np.zeros((C, NB, NR), np.int64)
    np.add.at(cnt, (core, blk, rid), 1)
    seg_cpb = -(-(cnt.max(axis=0) + 1) // P)           # [NB, NR] chunks

    # chunk numbering: for supergroup g: for r: for b in g: seg(b, r)
    n_groups = (NB + SG - 1) // SG
    seg_start = np.zeros((NB, NR), np.int64)
    calls = []          # (start_chunk, n_chunks, range_id)
    block_spans = {}    # b -> list of (c0, c1) in chunk order (per r)
    nch = 0
    for g in range(n_groups):
        bs = list(range(g * SG, min((g + 1) * SG, NB)))
        for r in range(NR):
            span0 = nch
            for b in bs:
                seg_start[b, r] = nch
                block_spans.setdefault(b, []).append(
                    (nch, nch + int(seg_cpb[b, r])))
                nch += int(seg_cpb[b, r])
            c0 = span0
            while c0 < nch:
                w = min(cfg.GK, nch - c0)
                calls.append((c0, w, r))
                c0 += w
    NCH = nch

    # edge -> (core, chunk, partition)
    key = (core.astype(np.int64) * NB + blk) * NR + rid
    order = np.argsort(key, kind="stable")
    ks = key[order]
    starts = np.searchsorted(ks, np.arange(C * NB * NR))
    rank = np.arange(E) - starts[ks]
    c_ = ks // (NB * NR)
    b_ = (ks // NR) % NB
    r_ = ks % NR
    chunk = seg_start[b_, r_] + rank // P
    part = rank % P
    e = order

    # host edge weights: wr = ew*cos(q*(ent+ccf)), wi = ew*sin(...)
    phase = np.float32(q) * (ent + ccf)
    wr = (ew * np.cos(phase)).astype(np.float32)
    wi = (ew * np.sin(phase)).astype(np.float32)
    lrow_f = lrow.astype(np.float32)

    # int16 gather indices for layer 2: position (chunk*128+part) ->
    # [pos%16, pos//16], replicated across the 8 16-partition groups
    gidxA = np.zeros((C, 16, NCH * 8), np.int16)
    pos = chunk * P + part
    piece_idx = src_c * QS + (src_l - rid * QS)        # index within piece
    gidxA[c_, pos % 16, pos // 16] = piece_idx[e].astype(np.int16)
    gidxA = np.tile(gidxA, (1, 8, 1))                  # [C, 128, NCH*8]

    tab = np.concatenate([real, imag], axis=1).astype(np.float16)  # [N, 128]

    def stk_a(W):
        H, Fd = W.shape
        out = np.zeros((2 * Fd, 2 * H), np.float16)
        out[:Fd, :H] = W.T
        out[Fd:, H:] = W.T
        return out

    def stk_b(W):
        H, Fd = W.shape
        out = np.zeros((2 * Fd, 2 * H), np.float16)
        out[Fd:, :H] = -W.T
        out[:Fd, H:] = W.T
        return out

    def brow(b):
        out = np.zeros((2 * len(b), 1), np.float32)
        out[len(b):, 0] = 2.0 * b
        return out

    consts = {
        "wa1": stk_a(W1), "wb1": stk_b(W1), "brow1": brow(b1),
        "wa2": stk_a(W2), "wb2": stk_b(W2), "brow2": brow(b2),
        "w3s": W3.T.astype(np.float16).copy(),           # [2H, O]
        "b3col": b3.astype(np.float32).reshape(-1, 1).copy(),
    }
    in_maps = []
    for c in range(cfg.CORES):
        m_e = c_ == c                        # edges of this core
        pa, ch = part[m_e], chunk[m_e]
        lr_c, ee = lrow[e][m_e], e[m_e]
        # weighted one-hot scatter masks [128, NCH, 256] f16
        maskc = np.zeros((P, NCH, 256), np.float16)
        maskc[pa, ch, lr_c] = wr[ee]
        maskc[pa, ch, P + lr_c] = wi[ee]
        # pre-gathered layer-1 edge source features [128, NCH, 128] f16
        g1c = np.zeros((P, NCH, P), np.float16)
        g1c[pa, ch, :] = tab[col[ee]]
        # compact per-chunk columns for on-device (DVE) mask building
        lwc = np.zeros((P, 3, NCH), np.float32)
        lwc[pa, 0, ch] = lrow_f[ee]
        lwc[pa, 1, ch] = wr[ee]
        lwc[pa, 2, ch] = wi[ee]
        m = {"g1": g1c, "mask": maskc, "gidx": gidxA[c], "lwc": lwc}
        m.update(consts)
        in_maps.append(m)
    meta = {"NCH": NCH, "calls": calls, "block_spans": block_spans,
            "n_groups": n_groups, "seg_cpb": seg_cpb}
    return in_maps, meta


def build_nc(cfg, meta):
    N, NPC, NB, GK, SG, NR, RSZ, GKS = (cfg.N, cfg.NPC, cfg.NB, cfg.GK,
                                        cfg.SG, cfg.NR, cfg.RSZ, cfg.GKS)
    NCH = meta["NCH"]
    calls = meta["calls"]
    block_spans = meta["block_spans"]
    n_groups = meta["n_groups"]
    O = 16
    NST = (NCH + GKS - 1) // GKS            # number of stream tiles
    NQ = int(os.environ.get('GNN_NQ', '4'))
    PREP = int(os.environ.get('GNN_PREP', '0'))
    W = int(os.environ.get('GNN_W', '128'))  # gather tile window (pool bufs)
    SCR = int(os.environ.get('GNN_SCR', '16384'))
    nc = bacc.Bacc(num_devices=cfg.CORES, num_swdge_queues=NQ,
                   dynamic_dma_scratch_size=SCR)

    g1_d = nc.declare_dram_parameter("g1", [P, NCH, P], F16, isOutput=False)
    mask_d = nc.declare_dram_parameter("mask", [P, NCH, 256], F16,
                                       isOutput=False)
    gidx_d = nc.declare_dram_parameter("gidx", [P, NCH * 8], I16,
                                       isOutput=False)
    lwc_d = nc.declare_dram_parameter("lwc", [P, 3, NCH], F32, isOutput=False)
    wa_d = [nc.declare_dram_parameter("wa1", [P, P], F16, isOutput=False),
            nc.declare_dram_parameter("wa2", [P, P], F16, isOutput=False)]
    wb_d = [nc.declare_dram_parameter("wb1", [P, P], F16, isOutput=False),
            nc.declare_dram_parameter("wb2", [P, P], F16, isOutput=False)]
    brow_d = [nc.declare_dram_parameter("brow1", [P, 1], F32, isOutput=False),
              nc.declare_dram_parameter("brow2", [P, 1], F32, isOutput=False)]
    w3s_d = nc.declare_dram_parameter("w3s", [P, O], F16, isOutput=False)
    b3_d = nc.declare_dram_parameter("b3col", [O, 1], F32, isOutput=False)
    out_t = nc.declare_dram_parameter("out_t", [O, NPC], F32, isOutput=True)

    QS = NPC // NR
    tab2in = [nc.dram_tensor(f"tab2in{p}", [QS, P], F16) for p in range(NR)]
    tabp = [nc.dram_tensor(f"tab2f{p}", [cfg.CORES * QS, P], F16,
                           addr_space="Shared") for p in range(NR)]
    # supergroup after which piece p of the layer-1 output is complete
    ag_sg = [(-(-((p + 1) * QS) // P) - 1) // SG for p in range(NR)]

    AluOp = mybir.AluOpType
    Act = mybir.ActivationFunctionType

    with tile.TileContext(nc) as tc:
        import contextlib
        with contextlib.ExitStack() as ctx:
            singles = ctx.enter_context(tc.tile_pool(name="singles", bufs=1))
            gpool = ctx.enter_context(tc.tile_pool(
                name="gpool", bufs=(W if PREP else 6)))
            mspool = ctx.enter_context(tc.tile_pool(name="mspool", bufs=3))
            g1pool = ctx.enter_context(tc.tile_pool(name="g1pool", bufs=3))
            p2pool = ctx.enter_context(tc.tile_pool(name="p2pool", bufs=2))
            lopool = ctx.enter_context(tc.tile_pool(name="lopool", bufs=2))
            twpool = ctx.enter_context(tc.tile_pool(name="twpool", bufs=2))
            topool = ctx.enter_context(tc.tile_pool(name="topool", bufs=2))
            pp_s = ctx.enter_context(tc.tile_pool(name="pp_s", bufs=4, space="PSUM"))
            pp_l = ctx.enter_context(tc.tile_pool(name="pp_l", bufs=2, space="PSUM"))
            pp_x = ctx.enter_context(tc.tile_pool(name="pp_x", bufs=2, space="PSUM"))

            # ---- resident metadata + constants ----
            gidx_s = singles.tile([P, NCH * 8], I16)
            nc.sync.dma_start(out=gidx_s, in_=gidx_d[:, :])
            DVEF = float(os.environ.get('GNN_DVEF', '0.35'))
            lwc_s = singles.tile([P, 3, NCH], F32)
            nc.sync.dma_start(out=lwc_s, in_=lwc_d[:, :, :])
            iota = singles.tile([P, P], F16)
            nc.gpsimd.iota(iota, pattern=[[1, P]], base=0,
                           channel_multiplier=0,
                           allow_small_or_imprecise_dtypes=True)

            wa = [singles.tile([P, P], F16, name=f"wa{i}") for i in range(2)]
            wb = [singles.tile([P, P], F16, name=f"wb{i}") for i in range(2)]
            brow = [singles.tile([P, 1], F32, name=f"brow{i}") for i in range(2)]
            for i in range(2):
                nc.sync.dma_start(out=wa[i], in_=wa_d[i][:, :])
                nc.sync.dma_start(out=wb[i], in_=wb_d[i][:, :])
                nc.sync.dma_start(out=brow[i], in_=brow_d[i][:, :])
            w3s = singles.tile([P, O], F16)
            nc.sync.dma_start(out=w3s, in_=w3s_d[:, :])
            b3c = singles.tile([O, 1], F32)
            nc.sync.dma_start(out=b3c, in_=b3_d[:, :])

            ident = singles.tile([P, P], F16)
            make_identity(nc, ident)

            # chunk -> (call index, offset within call)
            chunk_call = {}
            for ci, (c0, w, r) in enumerate(calls):
                for j in range(w):
                    chunk_call[c0 + j] = (ci, j)

            # Manual prep/trigger/wait protocol for layer-2 gathers:
            # descriptors are generated (prepare_only) ahead of the
            # AllGather data dependency; triggers fire per-call (count=1)
            # with at most PF calls of prefetch; consumers wait per-call
            # DMA-completion sems rotating over R sems (PF < R keeps the
            # cumulative 16-inc-per-call count an exact completion proof).
            R = 16
            PF = 12
            gsems = [nc.alloc_semaphore(f"gsem{i}") for i in range(R)]
            psems = [nc.alloc_semaphore(f"psem{q}") for q in range(NQ)]
            st = {"emitted": 0, "triggered": 0, "qpos": [0] * NQ,
                  "qfired": [0] * NQ}

            def emit_gather(g_tiles, prep):
                ci = st["emitted"]
                c0, w, r = calls[ci]
                q = ci % NQ
                gt = gpool.tile([P, GK, P], F16, tag="g", name=f"g_{ci}")
                g_tiles[ci] = gt
                ins = nc.gpsimd.dma_gather(
                    out_ap=gt[:, :w, :],
                    in_ap=tabp[r][0:, :],
                    idxs_ap=gidx_s[:, c0 * 8:(c0 + w) * 8],
                    num_idxs=w * P, num_idxs_reg=w * P,
                    elem_size=P, queue_num=q,
                    single_packet=os.environ.get('GNN_SP', '1') == '1',
                    prepare_only=bool(prep),
                    sem=gsems[ci % R] if prep else None)
                if prep:
                    ins.then_inc(psems[q], 1)
                st["qpos"][q] += 1
                st["emitted"] += 1

            def emit_trigger():
                ci = st["triggered"]
                q = ci % NQ
                st["qfired"][q] += 1
                nc.gpsimd.wait_ge(psems[q], st["qfired"][q])
                nc.gpsimd.trigger_dma(count=1, queue_num=q)
                st["triggered"] += 1

            # ---- two graph-conv layers ----
            for L in range(2):
                g_tiles = {}        # layer-2 gather tiles by call index
                ms_tiles = {}       # mask stream tiles by stream index
                g1_tiles = {}       # layer-1 G stream tiles by stream index

                def need_stream(c):
                    si = c // GKS
                    if si not in ms_tiles:
                        w = min(GKS, NCH - si * GKS)
                        mt = mspool.tile([P, GKS, 256], F16, tag="ms",
                                         name=f"ms{L}_{si}")
                        ms_tiles[si] = mt
                        if L == 0 and int((si + 1) * DVEF) > int(si * DVEF):
                            # build this tile's masks on the (idle) DVE:
                            # (iota==lrow)*wr | (iota==lrow)*wi per chunk
                            for cc in range(si * GKS, si * GKS + w):
                                j = cc - si * GKS
                                nc.vector.tensor_scalar(
                                    out=mt[:, j, 0:P], in0=iota[:, :],
                                    scalar1=lwc_s[:, 0, cc:cc + 1],
                                    scalar2=lwc_s[:, 1, cc:cc + 1],
                                    op0=AluOp.is_equal, op1=AluOp.mult)
                                nc.vector.tensor_scalar(
                                    out=mt[:, j, P:256], in0=iota[:, :],
                                    scalar1=lwc_s[:, 0, cc:cc + 1],
                                    scalar2=lwc_s[:, 2, cc:cc + 1],
                                    op0=AluOp.is_equal, op1=AluOp.mult)
                        else:
                            nc.sync.dma_start(
                                out=mt[:, :w, :],
                                in_=mask_d[:, si * GKS:si * GKS + w, :])
                        if L == 0:
                            g1t = g1pool.tile([P, GKS, P], F16, tag="g1",
                                              name=f"g1_{si}")
                            g1_tiles[si] = g1t
                            nc.scalar.dma_start(
                                out=g1t[:, :w, :],
                                in_=g1_d[:, si * GKS:si * GKS + w, :])
                    return si

                for g in range(n_groups):
                    bs = list(range(g * SG, min((g + 1) * SG, NB)))
                    first_chunk = block_spans[bs[0]][0][0]
                    last_chunk = block_spans[bs[-1]][-1][1]
                    if L == 1:
                        # gather calls: with PREP, emit prepare_only descs
                        # running AHEAD of consumption (window W tiles),
                        # triggers paced PF calls ahead of use
                        last_ci = max(ci for ci, (c0, w, r) in enumerate(calls)
                                      if first_chunk <= c0 < last_chunk)
                        GATE = int(os.environ.get('GNN_GATE', '3'))
                        if GATE >= 0 and st["emitted"] == 0:
                            # delay all desc-gen until piece GATE lands to
                            # keep Q7 off the SBUF port while DVE builds
                            gw = singles.tile([1, 2], F16, name="gw")
                            nc.gpsimd.dma_start(out=gw,
                                                in_=tabp[GATE][0:1, 0:2])
                        if PREP:
                            first = st["emitted"] == 0
                            tgt = min(len(calls), last_ci + 1 + (W - 32))
                            while st["emitted"] < tgt:
                                emit_gather(g_tiles, 1)
                            if first:
                                # tiny Pool-engine read of tab2f after the
                                # first prep batch: picks up the AllGather
                                # sync dep so every trigger (Pool,
                                # in-order) fires after the data arrives
                                agw = singles.tile([1, 2], F16, name="agw")
                                nc.gpsimd.dma_start(out=agw,
                                                    in_=tabp[NR - 1][0:1, 0:2])
                            tgt = min(st["emitted"], last_ci + 1 + PF)
                            while st["triggered"] < tgt:
                                emit_trigger()
                        else:
                            while st["emitted"] < last_ci + 1:
                                emit_gather(g_tiles, 0)
                    # one PSUM bank per block (sim tracks accumulation
                    # groups per bank; sharing a bank corrupts them)
                    pair = {}
                    for k in range(len(bs)):
                        pair[k] = pp_s.tile([P, 256], F32, space="PSUM",
                                            tag="ps", name=f"ps{L}_{g}_{k}")
                    blk_of = {}
                    blk_first = {}
                    blk_last = {}
                    for bi, b in enumerate(bs):
                        spans = block_spans[b]
                        blk_first[b] = spans[0][0]
                        blk_last[b] = spans[-1][1] - 1
                        for (c0, c1) in spans:
                            for c in range(c0, c1):
                                blk_of[c] = (bi, b)
                    for c in sorted(blk_of):
                        bi, b = blk_of[c]
                        psum = pair[bi]
                        si = need_stream(c)
                        rhs = ms_tiles[si][:, c - si * GKS, :]
                        if L == 0:
                            lhsT = g1_tiles[si][:, c - si * GKS, :]
                        else:
                            ci, j = chunk_call[c]
                            if PREP and j == 0:
                                # data-landing gate for this call's tile
                                nc.tensor.wait_ge(gsems[ci % R],
                                                  16 * (ci // R + 1))
                            lhsT = g_tiles[ci][:, j, :]
                        nc.tensor.matmul(
                            psum[:, 0:256],
                            lhsT=lhsT, rhs=rhs,
                            start=(c == blk_first[b]), stop=(c == blk_last[b]),
                            skip_group_check=True)
                    # finalize blocks
                    for bi, b in enumerate(bs):
                        psum = pair[bi]
                        p2c = p2pool.tile([P, 256], F16, tag="p2",
                                          name=f"p2_{L}_{b}")
                        nc.scalar.activation(out=p2c, in_=psum[:, 0:256],
                                             func=Act.Copy)
                        psl = pp_l.tile([P, P], F32, space="PSUM", tag="pl",
                                        name=f"pl{L}_{b}")
                        nc.tensor.matmul(psl[:, :], lhsT=wa[L], rhs=p2c[:, 0:P],
                                         start=True, stop=False)
                        nc.tensor.matmul(psl[:, :], lhsT=wb[L],
                                         rhs=p2c[:, P:256],
                                         start=False, stop=True)
                        lout = lopool.tile([P, P], F16, tag="lo",
                                           name=f"lo{L}_{b}")
                        nc.scalar.activation(out=lout, in_=psl, func=Act.Relu,
                                             bias=brow[L][:, 0:1])
                        nv = P if b < NB - 1 else cfg.NV_LAST
                        if L == 0:
                            pst = pp_x.tile([P, P], F16, space="PSUM",
                                            tag="px", name=f"px{b}")
                            nc.tensor.transpose(pst[:, :], lout[:, :],
                                                ident[:, :])
                            tblw = twpool.tile([P, P], F16, tag="tw",
                                               name=f"tw{b}")
                            nc.vector.tensor_copy(out=tblw, in_=pst)
                            # write node-major rows into piece tensors,
                            # splitting blocks that straddle a boundary
                            r0 = b * P
                            while r0 < b * P + nv:
                                p = r0 // QS
                                r1 = min(b * P + nv, (p + 1) * QS)
                                nc.sync.dma_start(
                                    out=tab2in[p][r0 - p * QS:r1 - p * QS, :],
                                    in_=tblw[r0 - b * P:r1 - b * P, :])
                                r0 = r1
                        else:
                            pso = pp_x.tile([P, P], F32, space="PSUM",
                                            tag="px", name=f"pxo{b}")
                            nc.tensor.matmul(pso[:O, :], lhsT=w3s[:, :],
                                             rhs=lout[:, :], start=True,
                                             stop=True)
                            osb = topool.tile([O, P], F32, tag="to",
                                              name=f"to{b}")
                            nc.scalar.activation(out=osb, in_=pso[:O, :],
                                                 func=Act.Identity,
                                                 bias=b3c[:, 0:1])
                            nc.sync.dma_start(out=out_t[:, b * P:b * P + nv],
                                              in_=osb[:, :nv])
                    if L == 0:
                        # fire each piece's AllGather as soon as its blocks
                        # are finalized — pieces 0..NR-2 overlap layer-1
                        # compute, and layer-2 desc-gen for early pieces
                        # can start before layer 1 finishes
                        for p in range(NR):
                            if ag_sg[p] == g:
                                nc.gpsimd.collective_compute(
                                    "AllGather", AluOp.bypass,
                                    replica_groups=[list(range(cfg.CORES))],
                                    ins=[tab2in[p].ap().opt()],
                                    outs=[tabp[p].ap().opt()],
                                )
    nc.compile()
    return nc


_CACHE = {}


def _get_nc(cfg, meta):
    key = (cfg.N, cfg.E, cfg.CORES, cfg.GK, cfg.SG, cfg.GKS,
           tuple(c for call in meta["calls"] for c in call))
    if key not in _CACHE:
        _CACHE[key] = build_nc(cfg, meta)
    return _CACHE[key]


def run(cfg, inputs, trace=False):
    from concourse.bass_utils import run_bass_kernel_spmd

    in_maps, meta = host_prep(
        cfg,
        np.asarray(inputs["real_feature"], np.float32),
        np.asarray(inputs["imag_feature"], np.float32),
        np.asarray(inputs["edge_weight_sym"], np.float32),
        np.float32(inputs["exp_weight_q"]),
        np.asarray(inputs["edge_entropy"], np.float32),
        np.asarray(inputs["edge_cluster_coefficient"], np.float32),
        np.asarray(inputs["W1"], np.float32), np.asarray(inputs["b1"], np.float32),
        np.asarray(inputs["W2"], np.float32), np.asarray(inputs["b2"], np.float32),
        np.asarray(inputs["W3"], np.float32), np.asarray(inputs["b3"], np.float32),
        np.asarray(inputs["row"]).astype(np.int64),
        np.asarray(inputs["col"]).astype(np.int64),
    )
    nc = _get_nc(cfg, meta)
    res = run_bass_kernel_spmd(nc, in_maps, list(range(cfg.CORES)), trace=trace)
    out = np.empty((cfg.N, 16), np.float32)
    for c in range(cfg.CORES):
        out[c * cfg.NPC:(c + 1) * cfg.NPC, :] = res.results[c]["out_t"].T
    return out, res


def kernel(**inputs) -> np.ndarray:
    cfg = Cfg(100000, 1000000, cores=8,
              gk=int(os.environ.get('GNN_GK', '3')),
              gks=int(os.environ.get('GNN_GKS', '24')))
    out, _ = run(cfg, inputs, trace=False)
    return out
